# revision 1
# baseline (speedup 1.0000x reference)
"""Conv-KNRM Trainium2 kernel (8 NeuronCores, data-parallel over batch).

Contract: kernel(**inputs) takes FULL inputs (as produced by the problem's
setup_inputs) and returns the FULL [32] float32 output. Everything heavy runs
on-device via a Bass program executed SPMD on 8 cores (4 examples per core).
"""

import math

import numpy as np

import concourse.bass as bass
import concourse.tile as tile
from concourse import bacc
from concourse import mybir
from concourse.bass_utils import run_bass_kernel_spmd

F32 = mybir.dt.float32
I32 = mybir.dt.int32
AF = mybir.ActivationFunctionType
ALU = mybir.AluOpType

# problem shapes (hardcoded; kernel.py must be self-contained)
B, LQ, LD = 32, 32, 256
V, E, C = 100000, 300, 128
G, K = 3, 11
NCORES = 8
EX = B // NCORES          # 4 examples per core
DL = LD + 2               # d positions + 2 zero pad cols for conv shifts
QL = 34                   # q positions needed per example (32 outputs + 2 lookahead)
QN = EX * QL              # 136 q gather rows per core
DN = EX * LD              # 1024 d gather rows per core
ECH = [(0, 128), (128, 128), (256, 44)]  # E=300 split for the 128-partition contract dim

# mus / sigmas of the 11 RBF kernels (must match reference.kernel_mus_sigmas)
def _mus_sigmas(n):
    bs = 2.0 / (n - 1)
    mus = [1.0, 1.0 - bs / 2.0]
    for i in range(1, n - 1):
        mus.append(mus[i] - bs)
    sig = [0.001] + [0.5 * bs] * (n - 1)
    return np.asarray(mus, np.float64), np.asarray(sig, np.float64)

MU, SIGMA = _mus_sigmas(K)
CINV = 1.0 / (2.0 * SIGMA**2)           # [K]; CINV[0]=5e5, CINV[k>=1]=50
BK = 2.0 * CINV * MU                    # b_k = 2*c*mu (only used k>=1)
CK = CINV * MU**2                       # c_k = c*mu^2
MASK_BIG = 1.0e4                        # added to masked m entries


def build_nc():
    nc = bacc.Bacc(None, target_bir_lowering=False)

    # ---- DRAM parameters -------------------------------------------------
    emb = nc.declare_dram_parameter("emb", [V, E], F32, isOutput=False)
    d_idx = nc.declare_dram_parameter("d_idx", [DN, 1], I32, isOutput=False)
    q_idx = nc.declare_dram_parameter("q_idx", [QN, 1], I32, isOutput=False)
    w_par = {}
    for g in range(1, G + 1):
        for j in range(g):
            w_par[(g, j)] = nc.declare_dram_parameter(f"w{g}{j}", [E, C], F32, isOutput=False)
    cb_par = [nc.declare_dram_parameter(f"cb{g}", [C, 1], F32, isOutput=False) for g in range(1, G + 1)]
    hmask_p = nc.declare_dram_parameter("hmask", [128, LD], F32, isOutput=False)
    qoov_p = nc.declare_dram_parameter("qoov", [128, EX], F32, isOutput=False)
    svec_p = nc.declare_dram_parameter("svec", [1, EX], F32, isOutput=False)
    thr_p = nc.declare_dram_parameter("thresh", [128, K], F32, isOutput=False)
    cneg_p = nc.declare_dram_parameter("cneg", [1, K], F32, isOutput=False)
    dw_p = nc.declare_dram_parameter("dw", [K, G * G], F32, isOutput=False)
    ones_p = nc.declare_dram_parameter("ones", [128, 1], F32, isOutput=False)
    ones1_p = nc.declare_dram_parameter("ones1", [1, 128], F32, isOutput=False)
    ident_p = nc.declare_dram_parameter("ident", [128, 128], F32, isOutput=False)
    out_p = nc.declare_dram_parameter("out", [1, EX], F32, isOutput=True)

    with tile.TileContext(nc) as tc:
        with (
            tc.tile_pool(name="persist", bufs=1) as pp,
            tc.tile_pool(name="work", bufs=2) as wp,
            tc.tile_pool(name="psum", bufs=2, space="PSUM") as ps,
        ):
            # ---- constants / small inputs into SBUF ----------------------
            ident = pp.tile([128, 128], F32, tag="ident")
            nc.sync.dma_start(out=ident[:], in_=ident_p[:])
            ones = pp.tile([128, 1], F32, tag="ones")
            nc.sync.dma_start(out=ones[:], in_=ones_p[:])
            ones1 = pp.tile([1, 128], F32, tag="ones1")
            nc.sync.dma_start(out=ones1[:], in_=ones1_p[:])
            hmask = pp.tile([128, LD], F32, tag="hmask")
            nc.sync.dma_start(out=hmask[:], in_=hmask_p[:])
            qoov = pp.tile([128, EX], F32, tag="qoov")
            nc.sync.dma_start(out=qoov[:], in_=qoov_p[:])
            svec = pp.tile([1, EX], F32, tag="svec")
            nc.sync.dma_start(out=svec[:], in_=svec_p[:])
            thr = pp.tile([128, K], F32, tag="thr")
            nc.sync.dma_start(out=thr[:], in_=thr_p[:])
            cneg = pp.tile([1, K], F32, tag="cneg")
            nc.sync.dma_start(out=cneg[:], in_=cneg_p[:])
            dw = pp.tile([K, G * G], F32, tag="dw")
            nc.sync.dma_start(out=dw[:], in_=dw_p[:])
            cb = []
            for g in range(1, G + 1):
                t = pp.tile([C, 1], F32, tag=f"cb{g}", name=f"cbs{g}")
                nc.sync.dma_start(out=t[:], in_=cb_par[g - 1][:])
                cb.append(t)
            wsb = {}
            for (g, j), par in w_par.items():
                for ci, (c0, cw) in enumerate(ECH):
                    t = pp.tile([cw, C], F32, tag=f"w{g}{j}c{ci}", name=f"w{g}{j}c{ci}")
                    nc.sync.dma_start(out=t[:], in_=par[c0 : c0 + cw, :])
                    wsb[(g, j, ci)] = t

            # ---- embedding gather + transpose to [E, L] ------------------
            # xd[ci]: [cw, EX*DL]  (cols ex*DL + t; t in [0,256) data, 256/257 zero)
            # xq[ci]: [cw, QN]     (cols ex*QL + p)
            xd = [pp.tile([cw, EX * DL], F32, tag=f"xd{ci}", name=f"xd{ci}") for ci, (_, cw) in enumerate(ECH)]
            xq = [pp.tile([cw, QN], F32, tag=f"xq{ci}", name=f"xq{ci}") for ci, (_, cw) in enumerate(ECH)]
            for t in xd:
                nc.vector.memzero(t[:])

            def gather_block(idx_param, row0, nrows):
                it = wp.tile([nrows, 1], I32, tag="idx")
                nc.sync.dma_start(out=it[:], in_=idx_param[row0 : row0 + nrows, :])
                gt = wp.tile([nrows, E], F32, tag="gt")
                nc.gpsimd.indirect_dma_start(
                    out=gt[:],
                    out_offset=None,
                    in_=emb[:],
                    in_offset=bass.IndirectOffsetOnAxis(ap=it[:, :1], axis=0),
                )
                return gt

            # d side: 8 blocks of 128 rows; block b covers ex=b//2, t0=(b%2)*128
            for b in range(DN // 128):
                gt = gather_block(d_idx, b * 128, 128)
                ex, t0 = b // 2, (b % 2) * 128
                for ci, (c0, cw) in enumerate(ECH):
                    tp = ps.tile([128, 128], F32, tag="tp")
                    nc.tensor.transpose(out=tp[:cw, :128], in_=gt[:, c0 : c0 + cw], identity=ident[:])
                    nc.vector.tensor_copy(
                        out=xd[ci][:, ex * DL + t0 : ex * DL + t0 + 128], in_=tp[:cw, :128]
                    )
            # q side: 128 rows + 8 rows
            gtA = gather_block(q_idx, 0, 128)
            gtB = gather_block(q_idx, 128, QN - 128)
            nb = QN - 128
            for ci, (c0, cw) in enumerate(ECH):
                tp = ps.tile([128, 128], F32, tag="tp")
                nc.tensor.transpose(out=tp[:cw, :128], in_=gtA[:, c0 : c0 + cw], identity=ident[:])
                nc.vector.tensor_copy(out=xq[ci][:, 0:128], in_=tp[:cw, :128])
                tp2 = ps.tile([128, 128], F32, tag="tp")
                nc.tensor.transpose(out=tp2[:cw, :nb], in_=gtB[:, c0 : c0 + cw], identity=ident[:nb, :nb])
                nc.vector.tensor_copy(out=xq[ci][:, 128:QN], in_=tp2[:cw, :nb])

            # ---- convolutions (matmuls over E, accumulating shifts) ------
            # d grams: dg[g-1] [C, EX*LD]; q grams: qg[g-1] [C, EX*32]
            dg = [pp.tile([C, EX * LD], F32, tag=f"dg{g}", name=f"dg{g}") for g in range(1, G + 1)]
            qg = [pp.tile([C, EX * 32], F32, tag=f"qg{g}", name=f"qg{g}") for g in range(1, G + 1)]

            for exp_ in range(2):           # example pairs (stationary reuse x2)
                exs = (2 * exp_, 2 * exp_ + 1)
                for g in range(1, G + 1):
                    acc = {ex: ps.tile([C, LD], F32, tag="acc", name=f"accd{exp_}{g}{ex}") for ex in exs}
                    steps = [(j, ci) for j in range(g) for ci in range(len(ECH))]
                    for si, (j, ci) in enumerate(steps):
                        c0, cw = ECH[ci]
                        for ex in exs:
                            nc.tensor.matmul(
                                out=acc[ex][:],
                                lhsT=wsb[(g, j, ci)][:],
                                rhs=xd[ci][:, ex * DL + j : ex * DL + j + LD],
                                start=(si == 0),
                                stop=(si == len(steps) - 1),
                            )
                    for ex in exs:
                        nc.vector.tensor_scalar(
                            out=dg[g - 1][:, ex * LD : (ex + 1) * LD],
                            in0=acc[ex][:],
                            scalar1=cb[g - 1][:],
                            scalar2=0.0,
                            op0=ALU.add,
                            op1=ALU.max,
                        )
            for exp_ in range(2):
                exs = (2 * exp_, 2 * exp_ + 1)
                for g in range(1, G + 1):
                    acc = {ex: ps.tile([C, 32], F32, tag="acc", name=f"accq{exp_}{g}{ex}") for ex in exs}
                    steps = [(j, ci) for j in range(g) for ci in range(len(ECH))]
                    for si, (j, ci) in enumerate(steps):
                        c0, cw = ECH[ci]
                        for ex in exs:
                            nc.tensor.matmul(
                                out=acc[ex][:],
                                lhsT=wsb[(g, j, ci)][:],
                                rhs=xq[ci][:, ex * QL + j : ex * QL + j + 32],
                                start=(si == 0),
                                stop=(si == len(steps) - 1),
                            )
                    for ex in exs:
                        nc.vector.tensor_scalar(
                            out=qg[g - 1][:, ex * 32 : (ex + 1) * 32],
                            in0=acc[ex][:],
                            scalar1=cb[g - 1][:],
                            scalar2=0.0,
                            op0=ALU.add,
                            op1=ALU.max,
                        )

            # ---- L2 normalization along C --------------------------------
            # inv = 1/(sqrt(ss)+1e-13), Newton-polished to rsqrt(ss) accuracy.
            def normalize(y, ncols, tagp):
                nch = (ncols + 127) // 128
                sq = wp.tile([C, ncols], F32, tag="n_sq")
                nc.vector.tensor_mul(sq[:], y[:], y[:])
                ss = ps.tile([C, nch], F32, tag="ssb")
                for ci in range(nch):
                    w = min(128, ncols - ci * 128)
                    nc.tensor.matmul(
                        out=ss[:w, ci : ci + 1],
                        lhsT=sq[:, ci * 128 : ci * 128 + w],
                        rhs=ones[:],
                        start=True,
                        stop=True,
                    )
                s = wp.tile([C, nch], F32, tag="n_s")
                nc.scalar.activation(out=s[:], in_=ss[:], func=AF.Sqrt)
                r0 = wp.tile([C, nch], F32, tag="n_r0")
                nc.vector.tensor_scalar_add(out=r0[:], in0=s[:], scalar1=1e-13)
                r1 = wp.tile([C, nch], F32, tag="n_r1")
                nc.vector.reciprocal(out=r1[:], in_=r0[:])
                # one Newton step towards rsqrt(ss): r = r1*(1.5 - 0.5*ss*r1^2)
                r2 = wp.tile([C, nch], F32, tag="n_r2")
                nc.vector.tensor_mul(r2[:], r1[:], r1[:])
                u = wp.tile([C, nch], F32, tag="n_u")
                nc.vector.tensor_mul(u[:], ss[:], r2[:])
                wv = wp.tile([C, nch], F32, tag="n_w")
                nc.vector.tensor_scalar(
                    out=wv[:], in0=u[:], scalar1=-0.5, scalar2=1.5, op0=ALU.mult, op1=ALU.add
                )
                inv = wp.tile([C, nch], F32, tag="n_inv")
                nc.vector.tensor_mul(inv[:], r1[:], wv[:])
                # per-chunk: transpose inv column -> [1,128] row, broadcast to
                # [128, w] via a K=1 matmul, then scale the gram columns.
                yn = pp.tile([C, ncols], F32, tag=f"yn{tagp}")
                for ci in range(nch):
                    w = min(128, ncols - ci * 128)
                    ivt_ps = ps.tile([1, 128], F32, tag="tp")
                    nc.tensor.transpose(out=ivt_ps[:, :], in_=inv[:, ci : ci + 1], identity=ident[:])
                    ivt = wp.tile([1, 128], F32, tag="n_ivt")
                    nc.vector.tensor_copy(out=ivt[:], in_=ivt_ps[:])
                    bc = ps.tile([C, 128], F32, tag="bcast")
                    nc.tensor.matmul(
                        out=bc[:, :w],
                        lhsT=ones1[:],
                        rhs=ivt[:, :w],
                        start=True,
                        stop=True,
                    )
                    nc.vector.tensor_mul(
                        yn[:, ci * 128 : ci * 128 + w],
                        y[:, ci * 128 : ci * 128 + w],
                        bc[:, :w],
                    )
                return yn

            dgn = [normalize(dg[g], EX * LD, f"d{g}") for g in range(G)]
            qgn = [normalize(qg[g], EX * 32, f"q{g}") for g in range(G)]

            # ---- pair loop: cosine matmul + RBF kernels + pooling --------
            fp_tiles = []
            sqrt_c0 = math.sqrt(CINV[0])
            for g in range(G):
                for h in range(G):
                    p = g * G + h
                    m_ps = ps.tile([128, LD], F32, tag="acc")
                    for ex in range(EX):
                        nc.tensor.matmul(
                            out=m_ps[ex * 32 : (ex + 1) * 32, :],
                            lhsT=qgn[g][:, ex * 32 : (ex + 1) * 32],
                            rhs=dgn[h][:, ex * LD : (ex + 1) * LD],
                            start=True,
                            stop=True,
                            tile_position=(0, 32 * ex),
                        )
                    mb = wp.tile([128, LD], F32, tag="mb")
                    nc.vector.tensor_add(mb[:], m_ps[:], hmask[:])
                    # Ein[:, k*LD:(k+1)*LD] holds Z_k with rk = exp(-Z_k)*e^{-c_k}
                    ein = wp.tile([128, K * LD], F32, tag="ein")
                    # k = 0 (exact-match kernel, c=5e5): Z_0 = c0*(m-1)^2
                    t1 = wp.tile([128, LD], F32, tag="t1")
                    nc.vector.tensor_scalar_add(out=t1[:], in0=mb[:], scalar1=-1.0)
                    nc.vector.scalar_tensor_tensor(
                        out=ein[:, 0:LD], in0=t1[:], scalar=float(CINV[0]),
                        in1=t1[:], op0=ALU.mult, op1=ALU.mult,
                    )
                    # sq50 = 50*m^2
                    sq50 = wp.tile([128, LD], F32, tag="sq50")
                    nc.vector.scalar_tensor_tensor(
                        out=sq50[:], in0=mb[:], scalar=float(CINV[1]),
                        in1=mb[:], op0=ALU.mult, op1=ALU.mult,
                    )
                    # k >= 1: Z_k = 50*m^2 - b_k*m   (split across DVE and GPSIMD)
                    for k in range(1, K):
                        eng = nc.vector
                        eng.scalar_tensor_tensor(
                            out=ein[:, k * LD : (k + 1) * LD],
                            in0=mb[:], scalar=float(-BK[k]),
                            in1=sq50[:], op0=ALU.mult, op1=ALU.add,
                        )
                    eb = wp.tile([128, K * LD], F32, tag="eb")
                    nc.scalar.activation(out=eb[:], in_=ein[:], func=AF.Exp, scale=-1.0)
                    perk = wp.tile([128, K], F32, tag="perk")
                    nc.vector.tensor_reduce(
                        out=perk[:],
                        in_=eb[:].rearrange("p (k t) -> p k t", k=K),
                        axis=mybir.AxisListType.X,
                        op=ALU.add,
                    )
                    pkc = wp.tile([128, K], F32, tag="pkc")
                    nc.vector.tensor_max(pkc[:], perk[:], thr[:])
                    logs = wp.tile([128, K], F32, tag="logs")
                    nc.scalar.activation(out=logs[:], in_=pkc[:], func=AF.Ln)
                    f_ps = ps.tile([K, EX], F32, tag="ssb")
                    nc.tensor.matmul(out=f_ps[:], lhsT=logs[:], rhs=qoov[:], start=True, stop=False)
                    nc.tensor.matmul(out=f_ps[:], lhsT=cneg[:], rhs=svec[:], start=False, stop=True)
                    fp = pp.tile([K, EX], F32, tag=f"fp{p}", name=f"fp{p}")
                    nc.vector.tensor_copy(out=fp[:], in_=f_ps[:])
                    fp_tiles.append(fp)

            # ---- dense + tanh -------------------------------------------
            o_ps = ps.tile([1, EX], F32, tag="ssb")
            for p in range(G * G):
                nc.tensor.matmul(
                    out=o_ps[:],
                    lhsT=dw[:, p : p + 1],
                    rhs=fp_tiles[p][:],
                    start=(p == 0),
                    stop=(p == G * G - 1),
                )
            ot = pp.tile([1, EX], F32, tag="ot")
            nc.scalar.activation(out=ot[:], in_=o_ps[:], func=AF.Tanh)
            nc.sync.dma_start(out=out_p[:], in_=ot[:])

    nc.compile()
    return nc


_NC_CACHE = None


def _get_nc():
    global _NC_CACHE
    if _NC_CACHE is None:
        _NC_CACHE = build_nc()
    return _NC_CACHE


def _host_inputs(q_tokens, d_tokens, emb_table, conv_ws, conv_bs, dense_w):
    """Build the per-core input maps."""
    emb = np.ascontiguousarray(emb_table, dtype=np.float32)
    shared = {"emb": emb}
    for g in range(1, G + 1):
        w = np.asarray(conv_ws[g - 1], np.float32)  # [C, E, g]
        for j in range(g):
            shared[f"w{g}{j}"] = np.ascontiguousarray(w[:, :, j].T)  # [E, C]
        shared[f"cb{g}"] = np.asarray(conv_bs[g - 1], np.float32).reshape(C, 1)
    shared["dw"] = np.ascontiguousarray(np.asarray(dense_w, np.float32).reshape(G * G, K).T)
    shared["ones"] = np.ones((128, 1), np.float32)
    shared["ones1"] = np.ones((1, 128), np.float32)
    shared["ident"] = np.eye(128, dtype=np.float32)
    thr_row = np.where(
        np.arange(K) == 0, 1e-10, 1e-10 * np.exp(np.minimum(CK, 80.0))
    ).astype(np.float32)
    thr_row[0] = 1e-10
    shared["thresh"] = np.tile(thr_row, (128, 1))
    cneg_row = -CK.copy()
    cneg_row[0] = 0.0
    shared["cneg"] = cneg_row.reshape(1, K).astype(np.float32)

    qt = np.asarray(q_tokens, np.int64)
    dt = np.asarray(d_tokens, np.int64)
    in_maps = []
    for core in range(NCORES):
        b0 = core * EX
        qc = qt[b0 : b0 + EX]
        dc = dt[b0 : b0 + EX]
        m = dict(shared)
        m["d_idx"] = dc.astype(np.int32).reshape(DN, 1)
        qp = np.zeros((EX, QL), np.int32)
        qp[:, :LQ] = qc
        m["q_idx"] = qp.reshape(QN, 1)
        hm = np.where(dc > 0, 0.0, MASK_BIG).astype(np.float32)       # [EX, LD]
        m["hmask"] = np.repeat(hm, 32, axis=0)                        # [128, LD]
        qo = np.zeros((128, EX), np.float32)
        oov = 0.01 * (qc > 1).astype(np.float32)                      # [EX, LQ]
        for e in range(EX):
            qo[e * 32 : (e + 1) * 32, e] = oov[e]
        m["qoov"] = qo
        m["svec"] = oov.sum(axis=1).reshape(1, EX).astype(np.float32)
        in_maps.append(m)
    return in_maps


def kernel(
    q_tokens, d_tokens, emb_table,
    conv_w1, conv_b1, conv_w2, conv_b2, conv_w3, conv_b3, dense_w,
):
    nc = _get_nc()
    in_maps = _host_inputs(
        q_tokens, d_tokens, emb_table,
        [conv_w1, conv_w2, conv_w3], [conv_b1, conv_b2, conv_b3], dense_w,
    )
    res = run_bass_kernel_spmd(nc, in_maps, core_ids=list(range(NCORES)))
    out = np.empty((B,), np.float32)
    for core in range(NCORES):
        out[core * EX : (core + 1) * EX] = np.asarray(res.results[core]["out"]).reshape(EX)
    return out

